# revision 1
# baseline (speedup 1.0000x reference)
"""Trainium2 Bass kernel for nn_DiffTopkNet (soft bitonic top-k).

Strategy
--------
Data parallel over 8 cores (32 batch rows each). Per core:

1. Forward pass over x [32, 512] through the 45 bitonic compare-swap
   layers, recording per-layer mixing coefficients
   g = s*arctan(10*d)/pi  (beta = 0.5 - g).  arctan over the full range
   is computed exactly via the branch-free identity
       arctan(z) = arctan(clamp(z,+-1)) - arctan(clamp(1/z,+-1)) + sign(1/z)*pi/4
   using the ACT engine's [-pi/2, pi/2] Arctan LUT.

2. Backward pass: the output is S . M_45 ... M_1 where each M_t is the
   pairwise row-mixing matrix; instead of evolving the full [512, 512]
   soft permutation (the reference does), evolve Y [16, 512] from the
   selector rows backward — 32x less work.  Per layer:
       dY = Y[:, off] - Y[:, base];  Y[:, base] += beta*dY; Y[:, off] -= beta*dY

Layout: SBUF partitions = 4 column-chunks x 32 batch rows, so every
layer with j <= 64 is a purely within-partition strided op.  The three
layers with j in {128, 256} cross chunks and use SBUF->SBUF DMA
partition moves.
"""

import numpy as np

BATCH, SIZE, K, NCORES = 256, 512, 16, 8
BC = BATCH // NCORES          # 32 batch rows per core
NL = 45                       # bitonic layers for n=512
PI = float(np.pi)
F32 = None                    # set after mybir import


def _layers():
    out = []
    k = 2
    while k <= SIZE:
        j = k // 2
        while j >= 1:
            out.append((k, j))
            j //= 2
        k *= 2
    return out


LAYERS = _layers()
SPECIALS = [t for t, (k, j) in enumerate(LAYERS) if j >= 128]  # [28, 36, 37]
SGN_COLS = NL * 64 + len(SPECIALS) * 128


def _sgn_table():
    """[128, SGN_COLS] f32: s/pi per (chunk-partition, compact pair index)."""
    sgn = np.ones((128, SGN_COLS), np.float32)
    for t, (k, j) in enumerate(LAYERS):
        if j > 64:
            continue
        m = np.arange(64)
        for c in range(4):
            base = c * 128 + (m // j) * 2 * j + (m % j)
            s = np.where((base & k) == 0, 1.0, -1.0) / np.pi
            sgn[c * 32:(c + 1) * 32, 64 * t:64 * t + 64] = s[None, :].astype(np.float32)
    for si, t in enumerate(SPECIALS):
        k, j = LAYERS[t]
        col = NL * 64 + 128 * si
        if j == 128:
            # bases are chunks 0 and 2 (partitions 0:32 and 64:96)
            for c, ps in ((0, slice(0, 32)), (2, slice(64, 96))):
                base = c * 128 + np.arange(128)
                s = np.where((base & k) == 0, 1.0, -1.0) / np.pi
                sgn[ps, col:col + 128] = s[None, :].astype(np.float32)
        else:  # j == 256: bases are chunks 0,1 (partitions 0:64)
            for c, ps in ((0, slice(0, 32)), (1, slice(32, 64))):
                base = c * 128 + np.arange(128)
                s = np.where((base & k) == 0, 1.0, -1.0) / np.pi
                sgn[ps, col:col + 128] = s[None, :].astype(np.float32)
    return sgn


def build_nc():
    import concourse.bacc as bacc
    import concourse.mybir as mybir
    from concourse import tile

    f32 = mybir.dt.float32
    f16 = mybir.dt.float16
    AT = mybir.ActivationFunctionType
    OP = mybir.AluOpType

    nc = bacc.Bacc("TRN2", target_bir_lowering=False, debug=False, num_devices=1)
    x_d = nc.dram_tensor("x", [BC, SIZE], f32, kind="ExternalInput")
    sg_d = nc.dram_tensor("sgn", [128, SGN_COLS], f32, kind="ExternalInput")
    y_d = nc.dram_tensor("y", [BC, K, SIZE], f32, kind="ExternalOutput")

    with tile.TileContext(nc) as tc:
        with tc.tile_pool(name="persist", bufs=1) as pp, \
             tc.tile_pool(name="scratch", bufs=3) as sp:
            xA = pp.tile([128, 128], f32)
            xB = pp.tile([128, 128], f32)
            yA = pp.tile([128, K * 128], f16)
            yB = pp.tile([128, K * 128], f16)
            yF = pp.tile([128, K * 128], f32)
            sgn_t = pp.tile([128, SGN_COLS], f32)
            g_norm = pp.tile([128, NL * 64], f32)
            g_spec = pp.tile([128, len(SPECIALS) * 128], f32)
            g16 = pp.tile([128, NL * 64], f16)
            g16s = pp.tile([128, len(SPECIALS) * 128], f16)

            nc.sync.dma_start(sgn_t[:], sg_d[:])
            nc.sync.dma_start(xA[:], x_d[:].rearrange("b (c i) -> c b i", c=4))

            xs = [xA, xB]

            def fwd_normal(t, j, src, dst):
                nb = 64 // j
                sv = src[:].rearrange("p (nb two j) -> p nb two j", two=2, j=j)
                dv = dst[:].rearrange("p (nb two j) -> p nb two j", two=2, j=j)
                u, v = sv[:, :, 0, :], sv[:, :, 1, :]
                d = sp.tile([128, 64], f32, name="d")
                dc = sp.tile([128, 64], f32, name="dc")
                r = sp.tile([128, 64], f32, name="r")
                rc = sp.tile([128, 64], f32, name="rc")
                Aa = sp.tile([128, 64], f32, name="Aa")
                Bb = sp.tile([128, 64], f32, name="Bb")
                Ss = sp.tile([128, 64], f32, name="Ss")
                t1 = sp.tile([128, 64], f32, name="t1")
                Gg = sp.tile([128, 64], f32, name="Gg")
                nw = sp.tile([128, 64], f32, name="nw")
                d_v = d[:].rearrange("p (nb j) -> p nb j", j=j)
                nw_v = nw[:].rearrange("p (nb j) -> p nb j", j=j)
                nc.vector.tensor_tensor(d_v, v, u, op=OP.subtract)
                nc.vector.tensor_scalar(dc[:], d[:], 0.1, -0.1, op0=OP.min, op1=OP.max)
                nc.vector.reciprocal(r[:], d[:])
                nc.vector.tensor_scalar(rc[:], r[:], 10.0, -10.0, op0=OP.min, op1=OP.max)
                nc.scalar.activation(Aa[:], dc[:], AT.Arctan, scale=10.0)
                nc.scalar.activation(Bb[:], rc[:], AT.Arctan, scale=0.1)
                nc.scalar.activation(Ss[:], rc[:], AT.Sign)
                nc.vector.tensor_tensor(t1[:], Aa[:], Bb[:], op=OP.subtract)
                nc.vector.scalar_tensor_tensor(Gg[:], Ss[:], PI / 4, t1[:],
                                               op0=OP.mult, op1=OP.add)
                gs = g_norm[:, 64 * t:64 * t + 64]
                nc.vector.tensor_tensor(gs, Gg[:], sgn_t[:, 64 * t:64 * t + 64], op=OP.mult)
                nc.vector.scalar_tensor_tensor(nw[:], gs, 0.5, d[:],
                                               op0=OP.subtract, op1=OP.mult)
                nc.scalar.copy(g16[:, 64 * t:64 * t + 64], gs)
                nc.vector.tensor_tensor(dv[:, :, 0, :], u, nw_v, op=OP.subtract)
                nc.vector.tensor_tensor(dv[:, :, 1, :], v, nw_v, op=OP.add)

            def fwd_special(si, t, j, src, dst):
                groups = ([(slice(0, 32), slice(32, 64)), (slice(64, 96), slice(96, 128))]
                          if j == 128 else [(slice(0, 64), slice(64, 128))])
                col = NL * 64 + 128 * si
                vt = sp.tile([128, 128], f32, name="vt")
                d = sp.tile([128, 128], f32, name="d_s")
                dc = sp.tile([128, 128], f32, name="dc_s")
                r = sp.tile([128, 128], f32, name="r_s")
                rc = sp.tile([128, 128], f32, name="rc_s")
                Aa = sp.tile([128, 128], f32, name="Aa_s")
                Bb = sp.tile([128, 128], f32, name="Bb_s")
                Ss = sp.tile([128, 128], f32, name="Ss_s")
                t1 = sp.tile([128, 128], f32, name="t1_s")
                Gg = sp.tile([128, 128], f32, name="Gg_s")
                nw = sp.tile([128, 128], f32, name="nw_s")
                nt = sp.tile([128, 128], f32, name="nt")
                for pu, pv in groups:
                    nc.sync.dma_start(vt[pu, :], src[pv, :])
                    nc.vector.tensor_tensor(d[pu, :], vt[pu, :], src[pu, :], op=OP.subtract)
                    nc.vector.tensor_scalar(dc[pu, :], d[pu, :], 0.1, -0.1, op0=OP.min, op1=OP.max)
                    nc.vector.reciprocal(r[pu, :], d[pu, :])
                    nc.vector.tensor_scalar(rc[pu, :], r[pu, :], 10.0, -10.0, op0=OP.min, op1=OP.max)
                    nc.scalar.activation(Aa[pu, :], dc[pu, :], AT.Arctan, scale=10.0)
                    nc.scalar.activation(Bb[pu, :], rc[pu, :], AT.Arctan, scale=0.1)
                    nc.scalar.activation(Ss[pu, :], rc[pu, :], AT.Sign)
                    nc.vector.tensor_tensor(t1[pu, :], Aa[pu, :], Bb[pu, :], op=OP.subtract)
                    nc.vector.scalar_tensor_tensor(Gg[pu, :], Ss[pu, :], PI / 4, t1[pu, :],
                                                   op0=OP.mult, op1=OP.add)
                    gs = g_spec[pu, 128 * si:128 * si + 128]
                    nc.vector.tensor_tensor(gs, Gg[pu, :], sgn_t[pu, col:col + 128], op=OP.mult)
                    nc.vector.scalar_tensor_tensor(nw[pu, :], gs, 0.5, d[pu, :],
                                                   op0=OP.subtract, op1=OP.mult)
                    nc.scalar.copy(g16s[pu, 128 * si:128 * si + 128], gs)
                    nc.vector.tensor_tensor(dst[pu, :], src[pu, :], nw[pu, :], op=OP.subtract)
                    nc.sync.dma_start(nt[pv, :], nw[pu, :])
                    nc.vector.tensor_tensor(dst[pv, :], src[pv, :], nt[pv, :], op=OP.add)

            for t, (k, j) in enumerate(LAYERS):
                src, dst = xs[t % 2], xs[(t + 1) % 2]
                if j <= 64:
                    fwd_normal(t, j, src, dst)
                else:
                    fwd_special(SPECIALS.index(t), t, j, src, dst)

            # ---- backward over Y [16 x 512] per batch row ----
            nc.vector.memset(yA[:], 0.0)
            nc.vector.memset(yA[:][96:128, 127:K * 128:127], 1.0)
            ys = [yA, yB]

            def bwd_normal(t, j, src, dst):
                nb = 64 // j
                sv = src[:].rearrange("p (k nb two j) -> p k nb two j", k=K, two=2, j=j)
                dv = dst[:].rearrange("p (k nb two j) -> p k nb two j", k=K, two=2, j=j)
                YU, YV = sv[:, :, :, 0, :], sv[:, :, :, 1, :]
                dY = sp.tile([128, K * 64], f16, name="dY")
                nwb = sp.tile([128, K * 64], f16, name="nwb")
                dY_v = dY[:].rearrange("p (k nb j) -> p k nb j", k=K, j=j)
                nwb_v = nwb[:].rearrange("p (k nb j) -> p k nb j", k=K, j=j)
                g_bc = (g16[:, 64 * t:64 * t + 64]
                        .rearrange("p (o nb j) -> p o nb j", o=1, j=j)
                        .broadcast_to([128, K, nb, j]))
                nc.vector.tensor_tensor(dY_v, YV, YU, op=OP.subtract)
                nc.vector.scalar_tensor_tensor(nwb_v, g_bc, 0.5, dY_v,
                                               op0=OP.subtract, op1=OP.mult)
                nc.vector.tensor_tensor(dv[:, :, :, 0, :], YU, nwb_v, op=OP.subtract)
                nc.vector.tensor_tensor(dv[:, :, :, 1, :], YV, nwb_v, op=OP.add)

            def bwd_sparse(t, j, nb0, nbc, tile_):
                # Support-limited early backward layers: only chunk-3
                # partitions and nbc blocks are nonzero; update in place.
                w = K * nbc * j
                sv = tile_[96:128, :].rearrange("p (k nb two j) -> p k nb two j",
                                                k=K, two=2, j=j)
                YU = sv[:, :, nb0:nb0 + nbc, 0, :]
                YV = sv[:, :, nb0:nb0 + nbc, 1, :]
                dY = sp.tile([128, K * 64], f16, name="dY")
                nwb = sp.tile([128, K * 64], f16, name="nwb")
                dY_v = dY[96:128, :w].rearrange("p (k nb j) -> p k nb j", k=K, j=j)
                nwb_v = nwb[96:128, :w].rearrange("p (k nb j) -> p k nb j", k=K, j=j)
                g_bc = (g16[96:128, 64 * t + nb0 * j:64 * t + (nb0 + nbc) * j]
                        .rearrange("p (o nb j) -> p o nb j", o=1, j=j)
                        .broadcast_to([32, K, nbc, j]))
                nc.vector.tensor_tensor(dY_v, YV, YU, op=OP.subtract)
                nc.vector.scalar_tensor_tensor(nwb_v, g_bc, 0.5, dY_v,
                                               op0=OP.subtract, op1=OP.mult)
                nc.vector.tensor_tensor(YU, YU, nwb_v, op=OP.subtract)
                nc.vector.tensor_tensor(YV, YV, nwb_v, op=OP.add)

            def bwd_special(si, t, j, src, dst):
                groups = ([(slice(0, 32), slice(32, 64)), (slice(64, 96), slice(96, 128))]
                          if j == 128 else [(slice(0, 64), slice(64, 128))])
                yvt = sp.tile([128, K * 128], f16, name="yvt")
                dY = sp.tile([128, K * 128], f16, name="dY_s")
                nwb = sp.tile([128, K * 128], f16, name="nwb_s")
                nyt = sp.tile([128, K * 128], f16, name="nyt")
                for pu, pv in groups:
                    L = pu.stop - pu.start
                    nc.sync.dma_start(yvt[pu, :], src[pv, :])
                    sv_u = src[pu, :].rearrange("p (k i) -> p k i", k=K)
                    vv = yvt[pu, :].rearrange("p (k i) -> p k i", k=K)
                    dY_v = dY[pu, :].rearrange("p (k i) -> p k i", k=K)
                    nwb_v = nwb[pu, :].rearrange("p (k i) -> p k i", k=K)
                    g_bc = (g16s[pu, 128 * si:128 * si + 128]
                            .rearrange("p (o i) -> p o i", o=1)
                            .broadcast_to([L, K, 128]))
                    nc.vector.tensor_tensor(dY_v, vv, sv_u, op=OP.subtract)
                    nc.vector.scalar_tensor_tensor(nwb_v, g_bc, 0.5, dY_v,
                                                   op0=OP.subtract, op1=OP.mult)
                    nc.vector.tensor_tensor(dst[pu, :], src[pu, :], nwb[pu, :], op=OP.subtract)
                    nc.sync.dma_start(nyt[pv, :], nwb[pu, :])
                    nc.vector.tensor_tensor(dst[pv, :], src[pv, :], nyt[pv, :], op=OP.add)

            # t -> (first block, n blocks) of the nonzero support
            SPARSE = {44: (56, 8), 43: (28, 4), 42: (14, 2),
                      41: (7, 1), 40: (3, 1), 39: (1, 1)}
            for t in range(NL - 1, NL - 1 - len(SPARSE), -1):
                k, j = LAYERS[t]
                nb0, nbc = SPARSE[t]
                bwd_sparse(t, j, nb0, nbc, yA)
            def bwd_special_zero(si, pu, pv, src, dst):
                # Cross-chunk layer where the base chunks (pu) are still all
                # zero: dY = v - 0 = v, base' = -negw, and any group whose
                # both sides are zero is skipped entirely (dst stays zero
                # from the init memset / prior zero-preserving writes).
                L = pu.stop - pu.start
                yvt = sp.tile([128, K * 128], f16, name="yvt")
                nwb = sp.tile([128, K * 128], f16, name="nwb_s")
                nyt = sp.tile([128, K * 128], f16, name="nyt")
                nc.sync.dma_start(yvt[pu, :], src[pv, :])
                vv = yvt[pu, :].rearrange("p (k i) -> p k i", k=K)
                nwb_v = nwb[pu, :].rearrange("p (k i) -> p k i", k=K)
                g_bc = (g16s[pu, 128 * si:128 * si + 128]
                        .rearrange("p (o i) -> p o i", o=1)
                        .broadcast_to([L, K, 128]))
                nc.vector.scalar_tensor_tensor(nwb_v, g_bc, 0.5, vv,
                                               op0=OP.subtract, op1=OP.mult)
                nc.vector.tensor_scalar_mul(dst[pu, :], nwb[pu, :], -1.0)
                nc.sync.dma_start(nyt[pv, :], nwb[pu, :])
                nc.vector.tensor_tensor(dst[pv, :], src[pv, :], nyt[pv, :], op=OP.add)

            s2 = 0
            for t in range(NL - 1 - len(SPARSE), -1, -1):
                k, j = LAYERS[t]
                src, dst = ys[s2 % 2], ys[(s2 + 1) % 2]
                s2 += 1
                if j <= 64:
                    bwd_normal(t, j, src, dst)
                elif t == 37:
                    # k=512, j=128: chunks 0,1 zero -> (c0,c1) group is a
                    # no-op (dst rows 0:64 already zero); (c2,c3) is zero-base
                    bwd_special_zero(SPECIALS.index(t), slice(64, 96),
                                     slice(96, 128), src, dst)
                elif t == 36:
                    # k=512, j=256: base chunks 0,1 zero
                    bwd_special_zero(SPECIALS.index(t), slice(0, 64),
                                     slice(64, 128), src, dst)
                else:
                    bwd_special(SPECIALS.index(t), t, j, src, dst)

            nc.vector.tensor_copy(yF[:], ys[(NL - len(SPARSE)) % 2][:])
            nc.sync.dma_start(y_d[:].rearrange("b k (c i) -> c b k i", c=4),
                              yF[:])

    nc.compile()
    return nc


_NC_CACHE = {}


def _get_nc():
    if "nc" not in _NC_CACHE:
        _NC_CACHE["nc"] = build_nc()
    return _NC_CACHE["nc"]


def _run_hw(vectors: np.ndarray) -> np.ndarray:
    from concourse.bass_utils import run_bass_kernel_spmd

    nc = _get_nc()
    sgn = _sgn_table()
    in_maps = [{"x": np.ascontiguousarray(vectors[c * BC:(c + 1) * BC]), "sgn": sgn}
               for c in range(NCORES)]
    res = run_bass_kernel_spmd(nc, in_maps, core_ids=list(range(NCORES)))
    out = np.empty((BATCH, K, SIZE), np.float32)
    for c in range(NCORES):
        out[c * BC:(c + 1) * BC] = res.results[c]["y"].reshape(BC, K, SIZE)
    return out


def _hw_worker(infile: str, outfile: str) -> None:
    vec = np.load(infile)
    np.save(outfile, _run_hw(vec))


def _run_sim(vectors: np.ndarray) -> np.ndarray:
    """Bit-exact local fallback (CoreSim) when the device path is unavailable."""
    from concourse.bass_interp import CoreSim

    nc = _get_nc()
    sgn = _sgn_table()
    out = np.empty((BATCH, K, SIZE), np.float32)
    for c in range(NCORES):
        sim = CoreSim(nc, require_finite=False, require_nnan=True)
        sim.tensor("x")[:] = vectors[c * BC:(c + 1) * BC]
        sim.tensor("sgn")[:] = sgn
        sim.simulate()
        out[c * BC:(c + 1) * BC] = np.array(sim.tensor("y")).reshape(BC, K, SIZE)
    return out


def kernel(vectors: np.ndarray) -> np.ndarray:
    import os
    import subprocess
    import sys
    import tempfile

    vectors = np.asarray(vectors, np.float32)
    assert vectors.shape == (BATCH, SIZE)

    # Hardware attempt in a watchdog subprocess: a wedged device tunnel can
    # hang an in-process PJRT execute forever; a subprocess we can time out.
    here = os.path.dirname(os.path.abspath(__file__))
    with tempfile.TemporaryDirectory() as td:
        inf = os.path.join(td, "in.npy")
        outf = os.path.join(td, "out.npy")
        np.save(inf, vectors)
        code = (
            "import sys; sys.path.insert(0, %r); "
            "import kernel; kernel._hw_worker(%r, %r)" % (here, inf, outf)
        )
        try:
            proc = subprocess.run(
                [sys.executable, "-c", code],
                timeout=int(os.environ.get("KERNEL_HW_TIMEOUT", "900")),
                capture_output=True,
            )
            if proc.returncode == 0 and os.path.exists(outf):
                return np.load(outf)
            sys.stderr.write(
                "kernel: hw subprocess failed (rc=%s); falling back to CoreSim\n%s\n"
                % (proc.returncode, proc.stderr.decode(errors="replace")[-2000:])
            )
        except subprocess.TimeoutExpired:
            sys.stderr.write("kernel: hw subprocess timed out; falling back to CoreSim\n")
    return _run_sim(vectors)



# revision 29
# speedup vs baseline: 2.4786x; 2.4786x over previous
"""Trainium2 Bass kernel for nn_DiffTopkNet (soft bitonic top-k).

Strategy
--------
Data parallel over 8 cores (32 batch rows each).  SBUF partitions =
4 column-chunks x 32 batch rows.  Per core:

1. Forward pass over x [32, 512] through the 45 bitonic compare-swap
   layers.  Per layer, the whole x-critical chain runs on the (cheap,
   in-order) Pool engine with a single merged Arctan on ACT:
       d = v - u;  args = [clamp(d,.1) | clamp(.01/d,.1)]
       AB = arctan(10*args);  t1 = AB_lo - AB_hi
       t2 = (d>=0)*pi/2 + t1;  t3 = (t2 - pi/4)*(d*sgnpi)
       dst_u = (u + d/2) - t3;  dst_v = (u + d/2) + t3
   (arctan(z) = arctan(clamp(z,+-1)) - arctan(clamp(1/z,+-1))
    + sign(z)*pi/4, valid for the ACT LUT range [-pi/2, pi/2].)
   The backward mixing coefficient beta16 = (t2-pi/4)*sgnpi - 0.5 is
   stored in f16, deferred into the next layer's ACT window.

2. Backward pass: the output is S . M_45 ... M_1; evolve Y [16, 512]
   from the selector rows backward (32x less work than the full soft
   permutation).  Per layer, 4 f16 tensor_tensor ops (2x DVE mode)
   k-split ~9/7 across DVE and Pool:
       dY = V - U;  nwb = beta*dY;  U' = U - nwb;  V' = V + nwb
   The first 6 layers touch only the 16..64-column support of the
   selector (in-place, Pool).  Cross-chunk layers t=37,36 exploit the
   zero support: U' = -beta*V, V' = (1+beta)*V with coefficients
   pre-staged on the right partitions during the forward pass; t=28
   moves the v-chunk via SBUF-SBUF DMA on the idle SP/ACT queues.
   The last layer writes f32 directly; the output DMA is split per
   (chunk, k-half) with batch-leading DRAM access patterns across the
   three DMA-capable engines.
"""

import numpy as np

BATCH, SIZE, K, NCORES = 256, 512, 16, 8
BC = BATCH // NCORES          # 32 batch rows per core
NL = 45                       # bitonic layers for n=512
PI = float(np.pi)
F32 = None                    # set after mybir import


def _layers():
    out = []
    k = 2
    while k <= SIZE:
        j = k // 2
        while j >= 1:
            out.append((k, j))
            j //= 2
        k *= 2
    return out


LAYERS = _layers()
SPECIALS = [t for t, (k, j) in enumerate(LAYERS) if j >= 128]  # [28, 36, 37]
SGN_COLS = NL * 64 + len(SPECIALS) * 128


def _sgn_table():
    """[128, SGN_COLS] f32: s/pi per (chunk-partition, compact pair index)."""
    sgn = np.ones((128, SGN_COLS), np.float16)
    for t, (k, j) in enumerate(LAYERS):
        if j > 64:
            continue
        m = np.arange(64)
        for c in range(4):
            base = c * 128 + (m // j) * 2 * j + (m % j)
            s = np.where((base & k) == 0, 1.0, -1.0) / np.pi
            sgn[c * 32:(c + 1) * 32, 64 * t:64 * t + 64] = s[None, :].astype(np.float16)
    for si, t in enumerate(SPECIALS):
        k, j = LAYERS[t]
        col = NL * 64 + 128 * si
        if j == 128:
            # bases are chunks 0 and 2 (partitions 0:32 and 64:96)
            for c, ps in ((0, slice(0, 32)), (2, slice(64, 96))):
                base = c * 128 + np.arange(128)
                s = np.where((base & k) == 0, 1.0, -1.0) / np.pi
                sgn[ps, col:col + 128] = s[None, :].astype(np.float16)
        else:  # j == 256: bases are chunks 0,1 (partitions 0:64)
            for c, ps in ((0, slice(0, 32)), (1, slice(32, 64))):
                base = c * 128 + np.arange(128)
                s = np.where((base & k) == 0, 1.0, -1.0) / np.pi
                sgn[ps, col:col + 128] = s[None, :].astype(np.float16)
    return sgn


def build_nc():
    import concourse.bacc as bacc
    import concourse.mybir as mybir
    from concourse import tile

    f32 = mybir.dt.float32
    f16 = mybir.dt.float16
    AT = mybir.ActivationFunctionType
    OP = mybir.AluOpType

    nc = bacc.Bacc("TRN2", target_bir_lowering=False, debug=False, num_devices=1)
    x_d = nc.dram_tensor("x", [BC, SIZE], f32, kind="ExternalInput")
    sg_d = nc.dram_tensor("sgn", [128, SGN_COLS], f16, kind="ExternalInput")
    y_d = nc.dram_tensor("y", [BC, K, SIZE], f32, kind="ExternalOutput")

    with tile.TileContext(nc) as tc:
        with tc.tile_pool(name="persist", bufs=1) as pp, \
             tc.tile_pool(name="scratch", bufs=3) as sp:
            xA = pp.tile([128, 128], f32)
            xB = pp.tile([128, 128], f32)
            yA = pp.tile([128, K * 128], f16)
            yB = pp.tile([128, K * 128], f16)
            yF = pp.tile([128, K * 128], f32)
            sgn_t = pp.tile([128, SGN_COLS], f16)
            b16 = pp.tile([128, NL * 64], f16)
            b16s = pp.tile([128, len(SPECIALS) * 128], f16)
            bneg16 = pp.tile([128, len(SPECIALS) * 128], f16)
            b1p16 = pp.tile([128, len(SPECIALS) * 128], f16)
            b1tmp = pp.tile([128, 128], f16)

            cent = pp.tile([128, 128], f32)
            nc.vector.memset(cent[:], 0.01)
            fwd_state = []
            nc.sync.dma_start(xA[:], x_d[:].rearrange("b (c i) -> c b i", c=4))
            H0 = 10 * 64
            H = SGN_COLS // 2
            nc.sync.dma_start(sgn_t[:, :H0], sg_d[:, :H0])
            nc.sync.dma_start(sgn_t[:, H0:H], sg_d[:, H0:H])
            nc.sync.dma_start(sgn_t[:, H:], sg_d[:, H:])

            xs = [xA, xB]

            def fwd_normal(t, j, src, dst):
                # Whole x-chain on Pool (53ns/op at 64 wide vs 127 on DVE),
                # ACT only for the single merged arctan over [dc | 0.01/d]:
                #   t1 = atan(10*dc) - atan(0.1/d);  t2 = b*pi/2 + t1
                #   t3 = (t2 - pi/4)*(d*sgnpi);  dst = (u + d/2) -/+ t3
                # Off-path ops (b, sd, sav, prev-layer beta) fill the ACT
                # round-trip window on the in-order Pool queue.
                sv = src[:].rearrange("p (nb two j) -> p nb two j", two=2, j=j)
                dv = dst[:].rearrange("p (nb two j) -> p nb two j", two=2, j=j)
                u, v = sv[:, :, 0, :], sv[:, :, 1, :]
                d = sp.tile([128, 64], f32, name="d")
                cs = sp.tile([128, 128], f32, name="cs")
                AB = sp.tile([128, 128], f32, name="AB")
                b01 = sp.tile([128, 64], f32, name="b01")
                sd = sp.tile([128, 64], f32, name="sd")
                sav = sp.tile([128, 64], f32, name="sav")
                t1 = sp.tile([128, 64], f32, name="t1")
                t2 = sp.tile([128, 64], f32, name="t2")
                t3 = sp.tile([128, 64], f32, name="t3")
                qa = sp.tile([128, 64], f32, name="qa")
                d_v = d[:].rearrange("p (nb j) -> p nb j", j=j)
                t3_v = t3[:].rearrange("p (nb j) -> p nb j", j=j)
                sav_v = sav[:].rearrange("p (nb j) -> p nb j", j=j)
                sg = sgn_t[:, 64 * t:64 * t + 64]
                nc.gpsimd.tensor_tensor(d_v, v, u, op=OP.subtract)
                nc.gpsimd.tensor_scalar(cs[:, :64], d[:], 0.1, -0.1, op0=OP.min, op1=OP.max)
                nc.gpsimd.tensor_tensor(cs[:, 64:], cent[:, :64], d[:], op=OP.divide)
                nc.gpsimd.tensor_scalar(cs[:, 64:], cs[:, 64:], 0.1, -0.1, op0=OP.min, op1=OP.max)
                nc.scalar.activation(AB[:], cs[:], AT.Arctan, scale=10.0)
                # fill the ACT window with off-path work
                nc.gpsimd.tensor_scalar(b01[:], d[:], 0.0, None, op0=OP.is_ge)
                nc.gpsimd.tensor_tensor(sd[:], d[:], sg, op=OP.mult)
                nc.gpsimd.scalar_tensor_tensor(sav_v, d_v, 0.5, u, op0=OP.mult, op1=OP.add)
                if t > 0:
                    pq = fwd_state.pop()
                    nc.gpsimd.scalar_tensor_tensor(pq[2], pq[0], -PI / 4, pq[1],
                                                   op0=OP.add, op1=OP.mult)
                    nc.gpsimd.tensor_scalar(pq[3], pq[2], -0.5, None, op0=OP.add)
                # post-ACT chain
                nc.gpsimd.tensor_tensor(t1[:], AB[:, :64], AB[:, 64:], op=OP.subtract)
                nc.gpsimd.scalar_tensor_tensor(t2[:], b01[:], PI / 2, t1[:],
                                               op0=OP.mult, op1=OP.add)
                nc.gpsimd.scalar_tensor_tensor(t3[:], t2[:], -PI / 4, sd[:],
                                               op0=OP.add, op1=OP.mult)
                nc.gpsimd.tensor_tensor(dv[:, :, 0, :], sav_v, t3_v, op=OP.subtract)
                nc.gpsimd.tensor_tensor(dv[:, :, 1, :], sav_v, t3_v, op=OP.add)
                # defer this layer's beta (qa = (t2-pi/4)*sgnpi; b16 = qa-0.5)
                # into the next layer's ACT window
                fwd_state.append((t2, sg, qa[:], b16[:, 64 * t:64 * t + 64]))
                if t == NL - 1:
                    pq = fwd_state.pop()
                    nc.gpsimd.scalar_tensor_tensor(pq[2], pq[0], -PI / 4, pq[1],
                                                   op0=OP.add, op1=OP.mult)
                    nc.gpsimd.tensor_scalar(pq[3], pq[2], -0.5, None, op0=OP.add)

            def fwd_special(si, t, j, src, dst):
                # Cross-chunk layer: v-chunk shuffled onto the u-partitions,
                # then the same Pool-chain as fwd_normal at [*,128]; results
                # for the v side and the backward coefficients are shuffled
                # back.  The two groups (j==128) are independent and
                # interleave.
                groups = ([(slice(0, 32), slice(32, 64)), (slice(64, 96), slice(96, 128))]
                          if j == 128 else [(slice(0, 64), slice(64, 128))])
                col = NL * 64 + 128 * si
                vt = sp.tile([128, 128], f32, name="vt")
                d = sp.tile([128, 128], f32, name="d_s")
                cs = sp.tile([128, 256], f32, name="cs_s")
                AB = sp.tile([128, 256], f32, name="AB_s")
                b01 = sp.tile([128, 128], f32, name="b01_s")
                sd = sp.tile([128, 128], f32, name="sd_s")
                sav = sp.tile([128, 128], f32, name="sav_s")
                t1 = sp.tile([128, 128], f32, name="t1_s")
                t2 = sp.tile([128, 128], f32, name="t2_s")
                t3 = sp.tile([128, 128], f32, name="t3_s")
                qa = sp.tile([128, 128], f32, name="qa_s")
                ntv = sp.tile([128, 128], f32, name="ntv")
                for pu, pv in groups:
                    nc.vector.stream_shuffle(vt[pu, :], src[pv, :], mask=list(range(32)))
                for gi, (pu, pv) in enumerate(groups):
                    sg = sgn_t[pu, col:col + 128]
                    ve = nc.gpsimd
                    ve.tensor_tensor(d[pu, :], vt[pu, :], src[pu, :], op=OP.subtract)
                    ve.tensor_scalar(cs[pu, :128], d[pu, :], 0.1, -0.1,
                                     op0=OP.min, op1=OP.max)
                    ve.tensor_tensor(cs[pu, 128:], cent[pu, :], d[pu, :], op=OP.divide)
                    ve.tensor_scalar(cs[pu, 128:], cs[pu, 128:], 0.1, -0.1,
                                     op0=OP.min, op1=OP.max)
                    nc.scalar.activation(AB[pu, :], cs[pu, :], AT.Arctan, scale=10.0)
                    ve.tensor_scalar(b01[pu, :], d[pu, :], 0.0, None, op0=OP.is_ge)
                    ve.tensor_tensor(sd[pu, :], d[pu, :], sg, op=OP.mult)
                    ve.scalar_tensor_tensor(sav[pu, :], d[pu, :], 0.5, src[pu, :],
                                            op0=OP.mult, op1=OP.add)
                    ve.tensor_tensor(t1[pu, :], AB[pu, :128], AB[pu, 128:],
                                     op=OP.subtract)
                    ve.scalar_tensor_tensor(t2[pu, :], b01[pu, :], PI / 2, t1[pu, :],
                                            op0=OP.mult, op1=OP.add)
                    ve.scalar_tensor_tensor(t3[pu, :], t2[pu, :], -PI / 4, sd[pu, :],
                                            op0=OP.add, op1=OP.mult)
                    ve.tensor_tensor(dst[pu, :], sav[pu, :], t3[pu, :], op=OP.subtract)
                    ve.tensor_tensor(ntv[pu, :], sav[pu, :], t3[pu, :], op=OP.add)
                    nc.gpsimd.scalar_tensor_tensor(qa[pu, :], t2[pu, :], -PI / 4, sg,
                                                    op0=OP.add, op1=OP.mult)
                    nc.gpsimd.tensor_scalar(b16s[pu, 128 * si:128 * si + 128], qa[pu, :],
                                            -0.5, None, op0=OP.add)
                    nc.gpsimd.tensor_scalar(bneg16[pu, 128 * si:128 * si + 128], qa[pu, :],
                                            -1.0, 0.5, op0=OP.mult, op1=OP.add)
                    nc.gpsimd.tensor_scalar(b1tmp[pu, :], qa[pu, :], 0.5, None, op0=OP.add)
                for pu, pv in groups:
                    nc.vector.stream_shuffle(dst[pv, :], ntv[pu, :], mask=list(range(32)))
                    nc.vector.stream_shuffle(b1p16[pv, 128 * si:128 * si + 128],
                                             b1tmp[pu, :], mask=list(range(32)))

            for t, (k, j) in enumerate(LAYERS):
                src, dst = xs[t % 2], xs[(t + 1) % 2]
                if j <= 64:
                    fwd_normal(t, j, src, dst)
                else:
                    fwd_special(SPECIALS.index(t), t, j, src, dst)

            # ---- backward over Y [16 x 512] per batch row ----
            nc.scalar.memzero(yA[:])
            nc.gpsimd.memset(yA[:][96:128, 127:K * 128:127], 1.0)
            ys = [yA, yB]

            KSPLIT = ((nc.vector, 0, 9), (nc.gpsimd, 9, K))

            def bwd_normal(t, j, src, dst):
                ksplit = ((nc.vector, 0, 8), (nc.gpsimd, 8, K)) if j == 1 else KSPLIT
                nb = 64 // j
                sv = src[:].rearrange("p (k nb two j) -> p k nb two j", k=K, two=2, j=j)
                dv = dst[:].rearrange("p (k nb two j) -> p k nb two j", k=K, two=2, j=j)
                dY = sp.tile([128, K * 64], f16, name="dY")
                nwb = sp.tile([128, K * 64], f16, name="nwb")
                dY_v = dY[:].rearrange("p (k nb j) -> p k nb j", k=K, j=j)
                nwb_v = nwb[:].rearrange("p (k nb j) -> p k nb j", k=K, j=j)
                for eng, klo, khi in ksplit:
                    kc = khi - klo
                    YU = sv[:, klo:khi, :, 0, :]
                    YV = sv[:, klo:khi, :, 1, :]
                    dYs = dY_v[:, klo:khi]
                    nwbs = nwb_v[:, klo:khi]
                    g_bc = (b16[:, 64 * t:64 * t + 64]
                            .rearrange("p (o nb j) -> p o nb j", o=1, j=j)
                            .broadcast_to([128, kc, nb, j]))
                    eng.tensor_tensor(dYs, YV, YU, op=OP.subtract)
                    eng.tensor_tensor(nwbs, g_bc, dYs, op=OP.mult)
                    eng.tensor_tensor(dv[:, klo:khi, :, 0, :], YU, nwbs, op=OP.subtract)
                    eng.tensor_tensor(dv[:, klo:khi, :, 1, :], YV, nwbs, op=OP.add)

            def bwd_sparse(t, j, nb0, nbc, tile_):
                # Support-limited early backward layers: only chunk-3
                # partitions and nbc blocks are nonzero; update in place.
                w = K * nbc * j
                sv = tile_[96:128, :].rearrange("p (k nb two j) -> p k nb two j",
                                                k=K, two=2, j=j)
                YU = sv[:, :, nb0:nb0 + nbc, 0, :]
                YV = sv[:, :, nb0:nb0 + nbc, 1, :]
                dY = sp.tile([128, K * 64], f16, name="dY")
                nwb = sp.tile([128, K * 64], f16, name="nwb")
                dY_v = dY[96:128, :w].rearrange("p (k nb j) -> p k nb j", k=K, j=j)
                nwb_v = nwb[96:128, :w].rearrange("p (k nb j) -> p k nb j", k=K, j=j)
                g_bc = (b16[96:128, 64 * t + nb0 * j:64 * t + (nb0 + nbc) * j]
                        .rearrange("p (o nb j) -> p o nb j", o=1, j=j)
                        .broadcast_to([32, K, nbc, j]))
                nc.gpsimd.tensor_tensor(dY_v, YV, YU, op=OP.subtract)
                nc.gpsimd.tensor_tensor(nwb_v, g_bc, dY_v, op=OP.mult)
                nc.gpsimd.tensor_tensor(YU, YU, nwb_v, op=OP.subtract)
                nc.gpsimd.tensor_tensor(YV, YV, nwb_v, op=OP.add)

            def bwd_special(si, t, j, src, dst):
                groups = ([(slice(0, 32), slice(32, 64)), (slice(64, 96), slice(96, 128))]
                          if j == 128 else [(slice(0, 64), slice(64, 128))])
                yvt = sp.tile([128, K * 128], f16, name="yvt")
                dY = sp.tile([128, K * 128], f16, name="dY_s")
                nwb = sp.tile([128, K * 128], f16, name="nwb_s")
                nyt = sp.tile([128, K * 128], f16, name="nyt")
                for gi, (pu, pv) in enumerate(groups):
                    (nc.sync if gi == 0 else nc.scalar).dma_start(
                        yvt[pu, :9 * 128], src[pv, :9 * 128])
                    (nc.scalar if gi == 0 else nc.sync).dma_start(
                        yvt[pu, 9 * 128:], src[pv, 9 * 128:])
                for pu, pv in groups:
                    L = pu.stop - pu.start
                    sv_u = src[pu, :].rearrange("p (k i) -> p k i", k=K)
                    vv = yvt[pu, :].rearrange("p (k i) -> p k i", k=K)
                    dY_v = dY[pu, :].rearrange("p (k i) -> p k i", k=K)
                    nwb_v = nwb[pu, :].rearrange("p (k i) -> p k i", k=K)
                    du_v = dst[pu, :].rearrange("p (k i) -> p k i", k=K)
                    for eng, klo, khi in KSPLIT:
                        kc = khi - klo
                        g_bc = (b16s[pu, 128 * si:128 * si + 128]
                                .rearrange("p (o i) -> p o i", o=1)
                                .broadcast_to([L, kc, 128]))
                        eng.tensor_tensor(dY_v[:, klo:khi], vv[:, klo:khi],
                                          sv_u[:, klo:khi], op=OP.subtract)
                        eng.tensor_tensor(nwb_v[:, klo:khi], g_bc,
                                          dY_v[:, klo:khi], op=OP.mult)
                        eng.tensor_tensor(du_v[:, klo:khi], sv_u[:, klo:khi],
                                          nwb_v[:, klo:khi], op=OP.subtract)
                for gi, (pu, pv) in enumerate(groups):
                    (nc.sync if gi == 0 else nc.scalar).dma_start(
                        nyt[pv, :9 * 128], nwb[pu, :9 * 128])
                    (nc.scalar if gi == 0 else nc.sync).dma_start(
                        nyt[pv, 9 * 128:], nwb[pu, 9 * 128:])
                for pu, pv in groups:
                    sv_v = src[pv, :].rearrange("p (k i) -> p k i", k=K)
                    dv_v = dst[pv, :].rearrange("p (k i) -> p k i", k=K)
                    ny_v = nyt[pv, :].rearrange("p (k i) -> p k i", k=K)
                    for eng, klo, khi in KSPLIT:
                        eng.tensor_tensor(dv_v[:, klo:khi], sv_v[:, klo:khi],
                                          ny_v[:, klo:khi], op=OP.add)

            # t -> (first block, n blocks) of the nonzero support
            SPARSE = {44: (56, 8), 43: (28, 4), 42: (14, 2),
                      41: (7, 1), 40: (3, 1), 39: (1, 1)}
            for t in range(NL - 1, NL - 1 - len(SPARSE), -1):
                k, j = LAYERS[t]
                nb0, nbc = SPARSE[t]
                bwd_sparse(t, j, nb0, nbc, yA)
            def bwd_special_zero(si, pu, pv, src, dst):
                # Base chunks (pu) all-zero:  newU = -beta*V (on pu, needs V
                # shuffled over),  newV = (1+beta)*V (in place on pv, with the
                # coefficient pre-staged on pv during the forward pass).
                L = pu.stop - pu.start
                yvt = sp.tile([128, K * 128], f16, name="yvt")
                nc.sync.dma_start(yvt[pu, :K * 64], src[pv, :K * 64])
                nc.scalar.dma_start(yvt[pu, K * 64:], src[pv, K * 64:])
                vv = yvt[pu, :].rearrange("p (k i) -> p k i", k=K)
                du_v = dst[pu, :].rearrange("p (k i) -> p k i", k=K)
                sv_v = src[pv, :].rearrange("p (k i) -> p k i", k=K)
                dv_v = dst[pv, :].rearrange("p (k i) -> p k i", k=K)
                for eng, klo, khi in KSPLIT:
                    kc = khi - klo
                    gneg = (bneg16[pu, 128 * si:128 * si + 128]
                            .rearrange("p (o i) -> p o i", o=1)
                            .broadcast_to([L, kc, 128]))
                    g1p = (b1p16[pv, 128 * si:128 * si + 128]
                           .rearrange("p (o i) -> p o i", o=1)
                           .broadcast_to([L, kc, 128]))
                    eng.tensor_tensor(dv_v[:, klo:khi], sv_v[:, klo:khi], g1p,
                                      op=OP.mult)
                    eng.tensor_tensor(du_v[:, klo:khi], vv[:, klo:khi], gneg,
                                      op=OP.mult)
            s2 = 0
            for t in range(NL - 1 - len(SPARSE), -1, -1):
                k, j = LAYERS[t]
                src, dst = ys[s2 % 2], ys[(s2 + 1) % 2]
                if t == 0:
                    dst = yF
                s2 += 1
                if j <= 64:
                    bwd_normal(t, j, src, dst)
                elif t == 37:
                    # k=512, j=128: chunks 0,1 zero -> (c0,c1) group is a
                    # no-op (dst rows 0:64 already zero); (c2,c3) is zero-base
                    bwd_special_zero(SPECIALS.index(t), slice(64, 96),
                                     slice(96, 128), src, dst)
                elif t == 36:
                    # k=512, j=256: base chunks 0,1 zero
                    bwd_special_zero(SPECIALS.index(t), slice(0, 64),
                                     slice(64, 128), src, dst)
                else:
                    bwd_special(SPECIALS.index(t), t, j, src, dst)

            # Output DMA split per (chunk, k-half): DRAM-side AP leads with
            # b=32 and keeps 512B contiguous runs; spreading across the three
            # DMA-capable engines overlaps the transfers.
            dma_engs = [nc.sync, nc.scalar, nc.gpsimd]
            di = 0
            for c in range(4):
                for kh in range(2):
                    dst = y_d[:, kh * 8:(kh + 1) * 8, c * 128:(c + 1) * 128]
                    src = (yF[32 * c:32 * (c + 1), :]
                           .rearrange("b (k i) -> b k i", k=K)[:, kh * 8:(kh + 1) * 8, :])
                    dma_engs[di % 3].dma_start(dst, src)
                    di += 1

    nc.compile()
    return nc


_NC_CACHE = {}


def _get_nc():
    if "nc" not in _NC_CACHE:
        _NC_CACHE["nc"] = build_nc()
    return _NC_CACHE["nc"]


def _run_hw(vectors: np.ndarray) -> np.ndarray:
    from concourse.bass_utils import run_bass_kernel_spmd

    nc = _get_nc()
    sgn = _sgn_table()
    in_maps = [{"x": np.ascontiguousarray(vectors[c * BC:(c + 1) * BC]), "sgn": sgn}
               for c in range(NCORES)]
    res = run_bass_kernel_spmd(nc, in_maps, core_ids=list(range(NCORES)))
    out = np.empty((BATCH, K, SIZE), np.float32)
    for c in range(NCORES):
        out[c * BC:(c + 1) * BC] = res.results[c]["y"].reshape(BC, K, SIZE)
    return out


def _hw_worker(infile: str, outfile: str) -> None:
    vec = np.load(infile)
    np.save(outfile, _run_hw(vec))


def _run_sim(vectors: np.ndarray) -> np.ndarray:
    """Bit-exact local fallback (CoreSim) when the device path is unavailable."""
    from concourse.bass_interp import CoreSim

    nc = _get_nc()
    sgn = _sgn_table()
    out = np.empty((BATCH, K, SIZE), np.float32)
    for c in range(NCORES):
        sim = CoreSim(nc, require_finite=False, require_nnan=True)
        sim.tensor("x")[:] = vectors[c * BC:(c + 1) * BC]
        sim.tensor("sgn")[:] = sgn
        sim.simulate()
        out[c * BC:(c + 1) * BC] = np.array(sim.tensor("y")).reshape(BC, K, SIZE)
    return out


def kernel(vectors: np.ndarray) -> np.ndarray:
    import os
    import subprocess
    import sys
    import tempfile

    vectors = np.asarray(vectors, np.float32)
    assert vectors.shape == (BATCH, SIZE)

    # Hardware attempt in a watchdog subprocess: a wedged device tunnel can
    # hang an in-process PJRT execute forever; a subprocess we can time out.
    here = os.path.dirname(os.path.abspath(__file__))
    with tempfile.TemporaryDirectory() as td:
        inf = os.path.join(td, "in.npy")
        outf = os.path.join(td, "out.npy")
        np.save(inf, vectors)
        code = (
            "import sys; sys.path.insert(0, %r); "
            "import kernel; kernel._hw_worker(%r, %r)" % (here, inf, outf)
        )
        try:
            proc = subprocess.run(
                [sys.executable, "-c", code],
                timeout=int(os.environ.get("KERNEL_HW_TIMEOUT", "900")),
                capture_output=True,
            )
            if proc.returncode == 0 and os.path.exists(outf):
                return np.load(outf)
            sys.stderr.write(
                "kernel: hw subprocess failed (rc=%s); falling back to CoreSim\n%s\n"
                % (proc.returncode, proc.stderr.decode(errors="replace")[-2000:])
            )
        except subprocess.TimeoutExpired:
            sys.stderr.write("kernel: hw subprocess timed out; falling back to CoreSim\n")
    return _run_sim(vectors)

